# revision 33
# baseline (speedup 1.0000x reference)
"""nn_Decoder (LSTM decoder) Trainium2 Bass kernel, 8-core tensor-parallel.

Strategy (hardcoded for B=64, L=128, H=1024, O=1, T=256, 8 cores):
  The LSTM recurrence is sequential in T, so per-step latency is everything.
  Instead of data-parallel batch sharding (which does not reduce per-step PE
  time at B<=128), the 4H=4096 gate rows are sharded 8 ways: each core owns a
  128-row H-slice of each gate (layout i|f|o|g), computes gates transposed
  [512, 64] on PE (W_hh^T blocks stationary in bf16, h^T streamed), does the
  cell elementwise on ACT/DVE in [128, 64] tiles, and broadcasts its 128-row
  h^T chunk (bf16, 16KB) to all peers each step.

  The exchange is ONE 8-destination remote_dma_broadcast per step per core
  (SBUF->SBUF direct, descriptor-count-optimal: 66 descriptors/engine/step).
  Each sender's GPSIMD stream branches once on its runtime core id (8-case
  Switch) so its chunk lands at slot <my_id> on every core (self included via
  loopback) -- pure logical-id addressing, independent of the physical NC
  permutation. x_gates is precomputed once and re-injected into each step's
  PSUM accumulation via identity matmuls (bf16 hi+lo split, error ~2^-18).
  The output projection (W_out is [1, H]) is one N=1 matmul per step into a
  dedicated PSUM bank; per-core partials are summed on the host.

  Measured on 8 axon trn2 cores: ~1.89 ms total (~7.4 us/step), rel err 2.1e-3.
"""

import numpy as np
import ml_dtypes

B, L, H, O, T = 64, 128, 1024, 1, 256
NC = 8
NPH = 4
# device gate-column order f|i|o|g (indices into pytorch's i,f,g,o row blocks):
# sigmoid(f,i) is the first ACT op so the DVE cell update starts earliest.
GATE_ORDER = [1, 0, 3, 2]
# gate column slices in the transposed [128, 256] gates tile
G_F = slice(0, 64)
G_I = slice(64, 128)
G_O = slice(128, 192)
G_G = slice(192, 256)
# keep-warm filler matmuls per round (PE HAM re-throttles to 1.2 GHz if idle
# >3.4us; the exchange leaves PE idle ~4.5us/round without these). Two blocks:
# one covering the elementwise window, one covering the exchange window.
FILLER_N = 256
FILLER_A = 14
FILLER_B = 0

_cache = {}


# --------------------------------------------------------------------------
# main kernel
# --------------------------------------------------------------------------


def _build_lstm_nc(T_steps=T, solo=False):
    """solo=True: replace the cross-core exchange with local SBUF->SBUF DMAs
    of identical shape (for single-core cost-model simulation)."""
    import concourse.bacc as bacc
    import concourse.bass as bass
    import concourse.mybir as mybir

    dt = mybir.dt
    AF = mybir.ActivationFunctionType
    ALU = mybir.AluOpType
    Tn = T_steps

    nc = bacc.Bacc(None, target_bir_lowering=False, debug=False, num_devices=NC)

    d_latT = nc.dram_tensor("latT", [128, 64], dt.float32, kind="ExternalInput")
    d_WlinT = nc.dram_tensor("WlinT", [128, 1024], dt.float32, kind="ExternalInput")
    d_blinT = nc.dram_tensor("blinT", [128, 8], dt.float32, kind="ExternalInput")
    d_WihT = nc.dram_tensor("WihT", [128, 4096], dt.float32, kind="ExternalInput")
    d_bg = nc.dram_tensor("bg", [1, 512], dt.float32, kind="ExternalInput")
    d_ones = nc.dram_tensor("ones1", [1, 64], dt.float32, kind="ExternalInput")
    d_WhT = nc.dram_tensor("WhT", [128, 4096], dt.bfloat16, kind="ExternalInput")
    d_wout = nc.dram_tensor("wout", [128, 1], dt.bfloat16, kind="ExternalInput")
    d_I64 = nc.dram_tensor("I64", [64, 64], dt.bfloat16, kind="ExternalInput")
    d_out = nc.dram_tensor("outp", [64, Tn], dt.float32, kind="ExternalOutput")
    N_IN = 9

    s_latT = nc.alloc_sbuf_tensor("s_latT", [128, 64], dt.float32)
    s_WlinT = nc.alloc_sbuf_tensor("s_WlinT", [128, 1024], dt.float32)
    s_blinT = nc.alloc_sbuf_tensor("s_blinT", [128, 8], dt.float32)
    s_WihT = nc.alloc_sbuf_tensor("s_WihT", [128, 4096], dt.float32)
    s_bg = nc.alloc_sbuf_tensor("s_bg", [1, 512], dt.float32)
    s_ones = nc.alloc_sbuf_tensor("s_ones", [1, 64], dt.float32)
    s_WhT = nc.alloc_sbuf_tensor("s_WhT", [128, 4096], dt.bfloat16)
    s_wout = nc.alloc_sbuf_tensor("s_wout", [128, 1], dt.bfloat16)
    s_I64 = nc.alloc_sbuf_tensor("s_I64", [64, 64], dt.bfloat16)

    s_hidT = nc.alloc_sbuf_tensor("s_hidT", [128, 512], dt.float32)
    s_Xhi = nc.alloc_sbuf_tensor("s_Xhi", [64, 512], dt.bfloat16)
    s_Xlo = nc.alloc_sbuf_tensor("s_Xlo", [64, 512], dt.bfloat16)
    s_Xres = nc.alloc_sbuf_tensor("s_Xres", [64, 512], dt.float32)
    recv = [
        nc.alloc_sbuf_tensor(f"recv{p}", [128, 512], dt.bfloat16) for p in range(NPH)
    ]
    s_gates = [
        nc.alloc_sbuf_tensor(f"s_gates{p}", [128, 256], dt.float32) for p in range(2)
    ]
    s_th = [nc.alloc_sbuf_tensor(f"s_th{p}", [128, 64], dt.float32) for p in range(2)]
    h_send = [
        nc.alloc_sbuf_tensor(f"h_send{p}", [128, 64], dt.bfloat16) for p in range(2)
    ]
    s_t1 = nc.alloc_sbuf_tensor("s_t1", [128, 64], dt.float32)
    s_t2 = nc.alloc_sbuf_tensor("s_t2", [128, 64], dt.float32)
    s_out = nc.alloc_sbuf_tensor("s_out", [64, Tn], dt.float32)

    p_hid = nc.alloc_psum_tensor("p_hid", [128, 512], dt.float32)
    p_x = nc.alloc_psum_tensor("p_x", [128, 512], dt.float32)
    p_g = [nc.alloc_psum_tensor(f"p_g{p}", [128, 512], dt.float32) for p in range(2)]
    p_out = nc.alloc_psum_tensor("p_out", [128, 512], dt.float32)
    p_fill = nc.alloc_psum_tensor("p_fill", [128, 512], dt.float32)
    p_c = nc.alloc_psum_tensor("p_c", [128, 512], dt.float32)
    s_c = [p_c[:, 0:64], p_c[:, 64:128]]  # cell state ping-pong (PSUM: faster ACT/DVE access)

    s_arr = [nc.alloc_semaphore(f"s_arr{p}") for p in range(NPH)]
    s_pe = nc.alloc_semaphore("s_pe")
    s_act = nc.alloc_semaphore("s_act")
    s_dve = nc.alloc_semaphore("s_dve")
    s_loc = nc.alloc_semaphore("s_loc")
    s_v = nc.alloc_semaphore("s_v")
    s_prep = nc.alloc_semaphore("s_prep")
    s_osem = nc.alloc_semaphore("s_osem")
    dma_sem = nc.alloc_semaphore("dma_sem")

    with nc.Block() as block:

        @block.sync
        def _(sync: bass.BassEngine):
            for d, s in [
                (d_latT, s_latT),
                (d_WlinT, s_WlinT),
                (d_blinT, s_blinT),
                (d_WihT, s_WihT),
                (d_bg, s_bg),
                (d_ones, s_ones),
                (d_WhT, s_WhT),
                (d_wout, s_wout),
                (d_I64, s_I64),
            ]:
                sync.dma_start(s[:, :], d[:, :]).then_inc(dma_sem, 16)
            sync.wait_ge(s_act, 4 * Tn + 2)
            sync.dma_start(d_out[:, :], s_out[:, :]).then_inc(dma_sem, 16)
            sync.wait_ge(dma_sem, 16 * (N_IN + 1))

        @block.tensor
        def _(tensor: bass.BassTensorEngine):
            tensor.wait_ge(dma_sem, 16 * N_IN)
            # phase 1a: hidden^T chunks = W_lin row-chunks @ latent^T
            for m in range(8):
                mm = tensor.matmul(
                    p_hid[:, 64 * m : 64 * m + 64],
                    s_WlinT[:, 128 * m : 128 * m + 128],
                    s_latT[:, :],
                    start=True,
                    stop=True,
                )
            mm.then_inc(s_pe, 1)  # s_pe = 1
            # phase 1b: x_gates (B-major) = hidden @ W_ih_slice^T + bias
            tensor.wait_ge(s_act, 1)
            for k in range(8):
                tensor.matmul(
                    p_x[0:64, :],
                    s_hidT[:, 64 * k : 64 * k + 64],
                    s_WihT[:, 512 * k : 512 * k + 512],
                    start=(k == 0),
                    stop=False,
                )
            mm = tensor.matmul(
                p_x[0:64, :], s_ones[0:1, :], s_bg[0:1, :], start=False, stop=True
            )
            mm.then_inc(s_pe, 1)  # s_pe = 2
            # one-time HAM warmup: >=3.4us of contiguous PE activity unthrottles
            # the PE clock 1.2 -> 2.4 GHz; the per-round fillers then keep it warm
            for fi in range(12):
                tensor.matmul(
                    p_fill[:, 0:512],
                    s_WhT[:, 0:128],
                    s_WhT[:, 128:640],
                    start=(fi == 0),
                    stop=(fi == 11),
                )

            for r in range(Tn):
                par = r % NPH
                pg = p_g[r % 2]
                if r == 0:
                    tensor.wait_ge(s_dve, 1)  # Xhi/Xlo ready
                if r >= 2:
                    tensor.wait_ge(s_act, 4 * r - 4)  # psum bank free
                for m in range(4):
                    tensor.matmul(
                        pg[:, 64 * m : 64 * m + 64],
                        s_Xhi[0:64, 128 * m : 128 * m + 128],
                        s_I64[0:64, :],
                        start=(m == 0),
                        stop=False,
                    )
                    mm = tensor.matmul(
                        pg[:, 64 * m : 64 * m + 64],
                        s_Xlo[0:64, 128 * m : 128 * m + 128],
                        s_I64[0:64, :],
                        start=False,
                        stop=(r == 0 and m == 3),
                    )
                # keep-warm fillers: PE would otherwise idle ~4.5us during the
                # elementwise + exchange and HAM re-throttles it to 1.2 GHz.
                # One accumulation group: per-MM start=True would cycle PSUM
                # groups and micro-idle the PE (the K18 HAM-oscillation trap).
                for fi in range(FILLER_A):
                    tensor.matmul(
                        p_fill[:, 0:FILLER_N],
                        s_WhT[:, 0:128],
                        s_WhT[:, 128 : 128 + FILLER_N],
                        start=(fi == 0),
                        stop=(fi == FILLER_A - 1),
                    )
                if r >= 1:
                    tensor.wait_ge(s_dve, 2 * r + 1)  # h_r in h_send[r%2]
                    tensor.matmul(
                        p_out[0:64, r - 1 : r],
                        h_send[r % 2][:, :],
                        s_wout[:, 0:1],
                        start=True,
                        stop=True,
                    )
                    for fi in range(FILLER_B):
                        tensor.matmul(
                            p_fill[:, 0:FILLER_N],
                            s_WhT[:, 0:128],
                            s_WhT[:, 128 : 128 + FILLER_N],
                            start=(fi == 0),
                            stop=(fi == FILLER_B - 1),
                        )
                    tensor.wait_ge(
                        s_arr[par], (128 if solo else 16) * ((r - 1) // NPH + 1)
                    )
                    for x in range(8):
                        for m in range(4):
                            mm = tensor.matmul(
                                pg[:, 64 * m : 64 * m + 64],
                                s_WhT[:, (4 * x + m) * 128 : (4 * x + m + 1) * 128],
                                recv[par][:, 64 * x : 64 * x + 64],
                                start=False,
                                stop=(x == 7 and m == 3),
                            )
                mm.then_inc(s_pe, 1)  # s_pe = 3 + r
            tensor.wait_ge(s_dve, 2 * Tn + 1)
            tensor.matmul(
                p_out[0:64, Tn - 1 : Tn],
                h_send[Tn % 2][:, :],
                s_wout[:, 0:1],
                start=True,
                stop=True,
            ).then_inc(s_osem, 1)

        @block.scalar
        def _(scalar: bass.BassScalarEngine):
            scalar.wait_ge(s_pe, 1)
            for m in range(8):
                a = scalar.activation(
                    s_hidT[:, 64 * m : 64 * m + 64],
                    p_hid[:, 64 * m : 64 * m + 64],
                    AF.Identity,
                    bias=s_blinT[:, m : m + 1],
                    scale=1.0,
                )
            a.then_inc(s_act, 1)  # s_act = 1
            for r in range(Tn):
                g = s_gates[r % 2]
                pg = p_g[r % 2]
                scalar.wait_ge(s_pe, 3 + r)
                scalar.activation(g[:, 0:128], pg[:, 0:128], AF.Sigmoid).then_inc(
                    s_act, 1
                )  # 4r+2  (f, i)
                scalar.activation(g[:, G_G], pg[:, G_G], AF.Tanh).then_inc(
                    s_act, 1
                )  # 4r+3  (g)
                scalar.activation(g[:, G_O], pg[:, G_O], AF.Sigmoid).then_inc(
                    s_act, 1
                )  # 4r+4  (o)
                scalar.wait_ge(s_dve, 2 * r + 2)
                scalar.activation(
                    s_th[r % 2][:, :], s_c[r % 2], AF.Tanh
                ).then_inc(s_act, 1)  # 4r+5
            scalar.wait_ge(s_osem, 1)
            scalar.activation(s_out[:, :], p_out[0:64, 0:Tn], AF.Copy).then_inc(
                s_act, 1
            )  # 4T+2

        @block.vector
        def _(vector: bass.BassVectorEngine):
            vector.wait_ge(s_pe, 2)
            vector.tensor_copy(s_Xhi[:, :], p_x[0:64, :]).then_inc(s_v, 1)  # 1
            vector.wait_ge(s_v, 1)
            vector.tensor_tensor(
                s_Xres[0:64, :], p_x[0:64, :], s_Xhi[:, :], ALU.subtract
            ).then_inc(s_v, 1)  # 2
            vector.wait_ge(s_v, 2)
            vector.tensor_copy(s_Xlo[:, :], s_Xres[0:64, :])
            vector.memset(s_c[1], 0.0).then_inc(s_dve, 1)  # s_dve = 1
            for r in range(Tn):
                g = s_gates[r % 2]
                if r == 0:
                    vector.wait_ge(s_dve, 1)
                vector.wait_ge(s_act, 4 * r + 2)
                vector.tensor_tensor(
                    s_t1[:, :], g[:, G_F], s_c[(r + 1) % 2], ALU.mult
                ).then_inc(s_v, 1)  # 3+2r
                vector.wait_ge(s_act, 4 * r + 3)
                vector.tensor_tensor(
                    s_t2[:, :], g[:, G_I], g[:, G_G], ALU.mult
                ).then_inc(s_v, 1)  # 4+2r
                vector.wait_ge(s_v, 4 + 2 * r)
                vector.tensor_tensor(
                    s_c[r % 2], s_t1[:, :], s_t2[:, :], ALU.add
                ).then_inc(s_dve, 1)  # 2r+2
                vector.wait_ge(s_act, 4 * r + 5)
                if r >= 2 and not solo:
                    # broadcast of round r-2 (which read h_send[(r+1)%2]) drained
                    vector.wait_ge(s_loc, 16 * (r - 1))
                vector.tensor_tensor(
                    h_send[(r + 1) % 2][:, :],
                    g[:, G_O],
                    s_th[r % 2][:, :],
                    ALU.mult,
                ).then_inc(s_dve, 1)  # 2r+3

        @block.gpsimd
        def _(gpsimd: bass.BassGpSimd):
            if solo:
                for r in range(Tn):
                    dst = recv[(r + 1) % NPH]
                    gpsimd.wait_ge(s_dve, 2 * r + 3)
                    for j in range(8):
                        gpsimd.dma_start(
                            dst[:, 64 * j : 64 * j + 64], h_send[(r + 1) % 2][:, :]
                        ).then_inc(s_arr[(r + 1) % NPH], 16)
                return
            gpsimd.bir_kernel_barrier_wait([list(range(NC))])
            pid = gpsimd.partition_id()
            for case in gpsimd.Switch(pid, NC):
                # one true 8-dest broadcast per round; my chunk lands at slot
                # `case` (my logical id) on every core, self included.
                for r in range(Tn):
                    dst = recv[(r + 1) % NPH]
                    gpsimd.remote_dma_broadcast(
                        out_ap=dst[:, 64 * case : 64 * case + 64],
                        in_ap=h_send[(r + 1) % 2][:, :],
                        remote_sem=s_arr[(r + 1) % NPH],
                        local_sem=s_loc,
                        rdests=[(0, j) for j in range(NC)],
                    ).then_inc(s_prep, 1)
                    gpsimd.wait_ge(s_prep, r + 1)
                    gpsimd.wait_ge(s_dve, 2 * r + 3)
                    gpsimd.trigger_dma(count=1)
                    gpsimd.wait_ge(s_loc, 16 * (r + 1))

    nc.has_collectives = not solo
    nc.finalize()
    return nc


def _prep_core_inputs(inputs: dict, r: int, src_row=None) -> dict:
    if src_row is None:
        src_row = list(range(8))  # slot j holds logical core j's H-chunk
    f32 = np.float32
    bf16 = ml_dtypes.bfloat16
    latent = np.asarray(inputs["latent"], f32)
    W_lin = np.asarray(inputs["W_lin"], f32)
    b_lin = np.asarray(inputs["b_lin"], f32)
    W_ih = np.asarray(inputs["W_ih"], f32)
    W_hh = np.asarray(inputs["W_hh"], f32)
    b_ih = np.asarray(inputs["b_ih"], f32)
    b_hh = np.asarray(inputs["b_hh"], f32)
    W_out = np.asarray(inputs["W_out"], f32)

    HS = 128
    sl = slice(HS * r, HS * (r + 1))

    Wih_sl = np.concatenate(
        [W_ih[g * H + HS * r : g * H + HS * (r + 1), :] for g in GATE_ORDER], axis=0
    )
    WihT = Wih_sl.T.reshape(8, 128, 512).transpose(1, 0, 2).reshape(128, 4096).copy()

    bgv = b_ih + b_hh
    bg = np.concatenate(
        [bgv[g * H + HS * r : g * H + HS * (r + 1)] for g in GATE_ORDER]
    ).reshape(1, 512)

    WhT = np.zeros((128, 4096), f32)
    for x in range(8):
        srcc = src_row[x]
        for m, g in enumerate(GATE_ORDER):
            blk = W_hh[
                g * H + HS * r : g * H + HS * (r + 1), HS * srcc : HS * (srcc + 1)
            ]
            WhT[:, (4 * x + m) * 128 : (4 * x + m + 1) * 128] = blk.T

    return {
        "latT": np.ascontiguousarray(latent.T),
        "WlinT": np.ascontiguousarray(W_lin.T),
        "blinT": np.ascontiguousarray(b_lin.reshape(8, 128).T),
        "WihT": WihT,
        "bg": bg,
        "ones1": np.ones((1, 64), f32),
        "WhT": WhT.astype(bf16),
        "wout": np.ascontiguousarray(W_out[0, sl].reshape(128, 1)).astype(bf16),
        "I64": np.eye(64, dtype=f32).astype(bf16),
    }


def _run(inputs: dict, trace: bool = False):
    from concourse.bass_utils import run_bass_kernel_spmd

    if "nc" not in _cache:
        _cache["nc"] = _build_lstm_nc(T)
    nc = _cache["nc"]
    in_maps = [_prep_core_inputs(inputs, r) for r in range(NC)]
    res = run_bass_kernel_spmd(
        nc, in_maps, core_ids=list(range(NC)), trace=trace
    )
    outs = [np.asarray(res.results[r]["outp"], np.float64) for r in range(NC)]
    b_out = np.asarray(inputs["b_out"], np.float64)
    total = outs[0]
    for o in outs[1:]:
        total = total + o
    total = total + b_out[0]
    out = total[:, :, None].astype(np.float32)
    return out, res


def kernel(**inputs) -> np.ndarray:
    seq_len = int(inputs.get("seq_len", T))
    assert seq_len == T, f"kernel hardcoded for seq_len={T}, got {seq_len}"
    out, _ = _run(inputs, trace=False)
    return out
